# revision 9
# baseline (speedup 1.0000x reference)
"""log_matmul_exp(x, A) on 8 TRN2 NeuronCores.

out[n, e] = logsumexp_d(x[n, d] + A[d, e]) = log(exp(x) @ exp(A))[n, e]

Inputs are standard-normal (|x|, |A| < ~6), so exp() spans ~[e-6, e6] and the
unshifted formulation is exact to fp32 rounding: no max-subtraction needed.

Sharding: 4 shards of N (rows of x / out) x 2 shards of E (cols of A / out).
x is transposed on the host so the contraction dim D sits on SBUF partitions
(fp32 has no DMA-transpose path on TRN2). Each core:
    exT = exp(xT_shard)  [D=1024, ML=1024]  (ACT, rounds to float32r)
    ea  = exp(A_shard)   [D=1024, EL=2048]  (ACT, rounds to float32r)
    s   = exT.T @ ea     (PE, float32r operands, fp32 PSUM accumulate)
    out = ln(s)          (ACT, fused into the PSUM->SBUF copyback)

DMA structure is constrained by walrus: a DMA descriptor supports only ONE
embedded sync wait, and Tile adds a lane-ordering wait whenever a DMA has a
predecessor on its round-robin DMA lane. So: input loads go to SWDGE lanes
(no data deps, lane wait only), and there are exactly 8 output stores (one
128-row slab each) on the 8 HWDGE lanes (data wait only).
"""

import os
import sys

import numpy as np

for _p in ("/opt/trn_rl_repo", "/root/.axon_site/_ro/trn_rl_repo"):
    if os.path.isdir(_p) and _p not in sys.path:
        sys.path.insert(0, _p)

P = 128
D = 1024
N_FULL = 4096
E_FULL = 4096
GRID_N = 4
GRID_E = 2
N_CORES = GRID_N * GRID_E
ML = N_FULL // GRID_N  # 1024 local output rows
EL = E_FULL // GRID_E  # 2048 local output cols
KC = D // P  # 8 contraction chunks
NT = 512  # matmul moving free dim (one PSUM bank of fp32)

_cache: dict = {}


def _build():
    import concourse.tile as tile
    from concourse import bacc, mybir

    AF = mybir.ActivationFunctionType
    f32 = mybir.dt.float32
    f32r = mybir.dt.float32r

    # Bacc (not raw Bass): its compile() runs generate_event_semaphores,
    # which splits multi-wait instructions to satisfy the 1-wait-per-
    # instruction hardware constraint that walrus codegen enforces.
    nc = bacc.Bacc(
        "TRN2",
        target_bir_lowering=False,
        debug=False,
        num_devices=N_CORES,
        num_swdge_queues=4,
        dynamic_dma_scratch_size=256,
    )
    xt = nc.dram_tensor("xt", [D, ML], f32, kind="ExternalInput")
    a = nc.dram_tensor("a", [D, EL], f32, kind="ExternalInput")
    out = nc.dram_tensor("out", [ML, EL], f32, kind="ExternalOutput")

    xt3 = xt[:].rearrange("(kc p) m -> p kc m", p=P)
    a3 = a[:].rearrange("(kc p) e -> p kc e", p=P)

    with tile.TileContext(nc) as tc:
        with (
            tc.tile_pool(name="persist", bufs=1) as persist,
            tc.tile_pool(name="outp", bufs=1) as outp,
            tc.tile_pool(name="psum", bufs=8, space="PSUM") as psum_pool,
            tc.tile_pool(name="stage", bufs=1) as stage,
        ):
            ex = []
            ea = []
            # Each stage tile is written by exactly one SWDGE DMA (no slot
            # reuse -> no WAR wait on the single descriptor wait slot).
            for kc in range(KC):
                st = stage.tile([P, ML], f32, tag=f"stx{kc}")
                nc.gpsimd.dma_start(st[:], xt3[:, kc])
                t = persist.tile([P, ML], f32r, tag=f"ex{kc}")
                nc.scalar.activation(t[:], st[:], AF.Exp)
                ex.append(t)
                su = stage.tile([P, EL], f32, tag=f"sta{kc}")
                nc.gpsimd.dma_start(su[:], a3[:, kc])
                u = persist.tile([P, EL], f32r, tag=f"ea{kc}")
                nc.scalar.activation(u[:], su[:], AF.Exp)
                ea.append(u)

            for mt in range(ML // P):
                ob = outp.tile([P, EL], f32)
                for nt in range(EL // NT):
                    ps = psum_pool.tile([P, NT], f32)
                    for kc in range(KC):
                        nc.tensor.matmul(
                            ps[:],
                            lhsT=ex[kc][:, mt * P : (mt + 1) * P],
                            rhs=ea[kc][:, nt * NT : (nt + 1) * NT],
                            start=(kc == 0),
                            stop=(kc == KC - 1),
                        )
                    nc.scalar.activation(ob[:, nt * NT : (nt + 1) * NT], ps[:], AF.Ln)
                # One store per 128-row output slab: 8 stores over 8 HWDGE
                # lanes, each with only its ACT data wait.
                nc.sync.dma_start(out[mt * P : (mt + 1) * P, :], ob[:])
    nc.compile()
    return nc


def _shard_inputs(x: np.ndarray, A: np.ndarray) -> list[dict]:
    xT = np.ascontiguousarray(x.T.astype(np.float32, copy=False))  # (D, N)
    A = np.asarray(A, dtype=np.float32)
    in_maps = []
    for c in range(N_CORES):
        i, j = divmod(c, GRID_E)
        in_maps.append(
            {
                "xt": np.ascontiguousarray(xT[:, i * ML : (i + 1) * ML]),
                "a": np.ascontiguousarray(A[:, j * EL : (j + 1) * EL]),
            }
        )
    return in_maps


def _run(x: np.ndarray, A: np.ndarray, trace: bool = False):
    from concourse import bass_utils

    nc = _cache.get("nc")
    if nc is None:
        nc = _build()
        _cache["nc"] = nc

    in_maps = _shard_inputs(np.asarray(x), np.asarray(A))
    res = bass_utils.run_bass_kernel_spmd(
        nc, in_maps, list(range(N_CORES)), trace=trace
    )
    out = np.empty((N_FULL, E_FULL), dtype=np.float32)
    for c in range(N_CORES):
        i, j = divmod(c, GRID_E)
        out[i * ML : (i + 1) * ML, j * EL : (j + 1) * EL] = res.results[c]["out"]
    return out, res


def kernel(x: np.ndarray, A: np.ndarray) -> np.ndarray:
    out, _ = _run(x, A, trace=False)
    return out


# revision 10
# speedup vs baseline: 1.4864x; 1.4864x over previous
"""log_matmul_exp(x, A) on 8 TRN2 NeuronCores.

out[n, e] = logsumexp_d(x[n, d] + A[d, e]) = log(exp(x) @ exp(A))[n, e]

Inputs are standard-normal (|x|, |A| < ~6), so exp() spans ~[e-6, e6] and the
unshifted formulation is exact to fp32 rounding: no max-subtraction needed.

Sharding: 4 shards of N (rows of x / out) x 2 shards of E (cols of A / out).
x is transposed on the host so the contraction dim D sits on SBUF partitions
(fp32 has no DMA-transpose path on TRN2). Each core:
    exT = exp(xT_shard)  [D=1024, ML=1024]  (ACT, rounds to float32r)
    ea  = exp(A_shard)   [D=1024, EL=2048]  (ACT, rounds to float32r)
    s   = exT.T @ ea     (PE, float32r operands, fp32 PSUM accumulate)
    out = ln(s)          (ACT, fused into the PSUM->SBUF copyback)

DMA structure is constrained by walrus: a DMA descriptor supports only ONE
embedded sync wait, and Tile adds a lane-ordering wait whenever a DMA has a
predecessor on its round-robin DMA lane. So: input loads go to SWDGE lanes
(no data deps, lane wait only), and there are exactly 8 output stores (one
128-row slab each) on the 8 HWDGE lanes (data wait only).
"""

import os
import sys

import numpy as np

for _p in ("/opt/trn_rl_repo", "/root/.axon_site/_ro/trn_rl_repo"):
    if os.path.isdir(_p) and _p not in sys.path:
        sys.path.insert(0, _p)

P = 128
D = 1024
N_FULL = 4096
E_FULL = 4096
GRID_N = 4
GRID_E = 2
N_CORES = GRID_N * GRID_E
ML = N_FULL // GRID_N  # 1024 local output rows
EL = E_FULL // GRID_E  # 2048 local output cols
KC = D // P  # 8 contraction chunks
NT = 512  # matmul moving free dim (one PSUM bank of fp32)

_cache: dict = {}


def _build():
    import concourse.tile as tile
    from concourse import bacc, mybir

    AF = mybir.ActivationFunctionType
    f32 = mybir.dt.float32
    f32r = mybir.dt.float32r

    # Bacc (not raw Bass): its compile() runs generate_event_semaphores,
    # which splits multi-wait instructions to satisfy the 1-wait-per-
    # instruction hardware constraint that walrus codegen enforces.
    nc = bacc.Bacc(
        "TRN2",
        target_bir_lowering=False,
        debug=False,
        num_devices=N_CORES,
        num_swdge_queues=4,
        dynamic_dma_scratch_size=256,
    )
    xt = nc.dram_tensor("xt", [D, ML], f32, kind="ExternalInput")
    a = nc.dram_tensor("a", [D, EL], f32, kind="ExternalInput")
    out = nc.dram_tensor("out", [ML, EL], f32, kind="ExternalOutput")

    xt3 = xt[:].rearrange("(kc p) m -> p kc m", p=P)
    a3 = a[:].rearrange("(kc p) e -> p kc e", p=P)

    MT = ML // P  # 8 row tiles
    ET = EL // NT  # 4 col tiles
    KH = KC // 2  # split-k: group 0 = kc 0..3, group 1 = kc 4..7

    with tile.TileContext(nc) as tc:
        with (
            tc.tile_pool(name="persist", bufs=1) as persist,
            tc.tile_pool(name="partial", bufs=1) as partial,
            tc.tile_pool(name="outp", bufs=2) as outp,
            tc.tile_pool(name="psum", bufs=8, space="PSUM") as psum_pool,
            tc.tile_pool(name="stage", bufs=3) as stage,
        ):
            ex = []
            ea = []
            for kc in range(KC):
                st = stage.tile([P, ML], f32, tag="stx")
                nc.sync.dma_start(st[:], xt3[:, kc])
                t = persist.tile([P, ML], f32r, tag=f"ex{kc}")
                nc.scalar.activation(t[:], st[:], AF.Exp)
                ex.append(t)
                su = stage.tile([P, EL], f32, tag="sta")
                nc.sync.dma_start(su[:], a3[:, kc])
                u = persist.tile([P, EL], f32r, tag=f"ea{kc}")
                nc.scalar.activation(u[:], su[:], AF.Exp)
                ea.append(u)

            # Split-k so the PE has work proportional to every arriving input
            # chunk (32 output tiles in flight) instead of stalling on the
            # full k-depth of a single 8-bank PSUM working set.
            parts = {}
            for mt in range(MT):
                for nt in range(ET):
                    ps = psum_pool.tile([P, NT], f32)
                    for kc in range(KH):
                        nc.tensor.matmul(
                            ps[:],
                            lhsT=ex[kc][:, mt * P : (mt + 1) * P],
                            rhs=ea[kc][:, nt * NT : (nt + 1) * NT],
                            start=(kc == 0),
                            stop=(kc == KH - 1),
                        )
                    pt = partial.tile([P, NT], f32, tag=f"pt{mt}_{nt}")
                    nc.vector.tensor_copy(pt[:], ps[:])
                    parts[mt, nt] = pt

            for mt in range(MT):
                ob = outp.tile([P, EL], f32)
                for nt in range(ET):
                    ps = psum_pool.tile([P, NT], f32)
                    for kc in range(KH, KC):
                        nc.tensor.matmul(
                            ps[:],
                            lhsT=ex[kc][:, mt * P : (mt + 1) * P],
                            rhs=ea[kc][:, nt * NT : (nt + 1) * NT],
                            start=(kc == KH),
                            stop=(kc == KC - 1),
                        )
                    pt = parts[mt, nt]
                    nc.vector.tensor_add(pt[:], ps[:], pt[:])
                    nc.scalar.activation(ob[:, nt * NT : (nt + 1) * NT], pt[:], AF.Ln)
                nc.sync.dma_start(out[mt * P : (mt + 1) * P, :], ob[:])
    nc.compile()
    return nc


def _shard_inputs(x: np.ndarray, A: np.ndarray) -> list[dict]:
    xT = np.ascontiguousarray(x.T.astype(np.float32, copy=False))  # (D, N)
    A = np.asarray(A, dtype=np.float32)
    in_maps = []
    for c in range(N_CORES):
        i, j = divmod(c, GRID_E)
        in_maps.append(
            {
                "xt": np.ascontiguousarray(xT[:, i * ML : (i + 1) * ML]),
                "a": np.ascontiguousarray(A[:, j * EL : (j + 1) * EL]),
            }
        )
    return in_maps


def _run(x: np.ndarray, A: np.ndarray, trace: bool = False):
    from concourse import bass_utils

    nc = _cache.get("nc")
    if nc is None:
        nc = _build()
        _cache["nc"] = nc

    in_maps = _shard_inputs(np.asarray(x), np.asarray(A))
    res = bass_utils.run_bass_kernel_spmd(
        nc, in_maps, list(range(N_CORES)), trace=trace
    )
    out = np.empty((N_FULL, E_FULL), dtype=np.float32)
    for c in range(N_CORES):
        i, j = divmod(c, GRID_E)
        out[i * ML : (i + 1) * ML, j * EL : (j + 1) * EL] = res.results[c]["out"]
    return out, res


def kernel(x: np.ndarray, A: np.ndarray) -> np.ndarray:
    out, _ = _run(x, A, trace=False)
    return out
